# revision 64
# baseline (speedup 1.0000x reference)
"""MoE FFN (8 experts, top-2, GLU) on 8 Trainium2 NeuronCores.

Strategy
--------
Phase 1 (on-device, data-parallel over tokens): each core computes router
logits for its 512-token shard in true fp32 (top-2 selection is
flip-sensitive: bf16 logits give 0.4% top-2 flips = 5e-2 output error),
then softmax-free gate weights c[t, e] = z_e / (z_1 + z_2) with
z = exp(logit) (the softmax denominator and max-shift cancel in the
ratio).  Exactly matches softmax + top-k + L1-normalize.

Host dispatch (data movement only): for each expert, gather the columns of
x^T (pre-cast to bf16) for its routed tokens into a fixed-capacity buffer.

Phase 2 (on-device, expert-parallel, bf16): core e computes the GLU FFN of
expert e over its gathered tokens.  bf16 runs at the same PE rate as
fp32r but halves every DMA (ramp shrinks 6.6us -> ~2.8us) and was
measured at 4.1e-3 end-to-end error on CPU (gate 2e-2).
    Phase A: h = silu(w1t^T xg) * (v1t^T xg)     [F, CA] bf16
    Phase B (H-major, w2 stationary):  yT = (w2^T h) . cb   [H, CA]
H-major Phase B streams exactly CA token columns (token-major padded to
ceil128(CA) = 8% waste); the per-token gate c is applied to the output
tiles via a host-broadcast [128, CA] tile.

Host combine (data movement only): out[idx_e] += yT_e[:, :n].T.

Scheduling notes (from timeline-sim traces): one shared HWDGE
descriptor generator serializes DMAs at ~625ns each across all hwdge
queues, so xg/cbs/w2 ride the gpsimd SWDGE (parallel generator); every
DMA-dependent consumer pays ~900ns semaphore propagation; the PE clocks
0.65/1.2 GHz until ~3us of continuous execution, so ~13 dummy matmuls
on a zeroed tile burn the ramp while the first DMAs land.

Measured (seed-0 inputs, 8 cores): relative error 4.7e-3 vs the fp32
reference; timeline-sim 194045 ns total (router 14247 + expert 179798
at CA=1063 exact; expert PE-busy ~173 us ~= the bf16 matmul roofline
170 us + warmup; vs 206791 ns for the fp32r token-major baseline).
"""

import numpy as np
import ml_dtypes

import concourse.bacc as bacc
import concourse.mybir as mybir
import concourse.tile as tile
from concourse.bass_utils import run_bass_kernel_spmd

P = 128
E = 8
H = 1024
F = 2048
T = 4096
NCORES = 8
TSH = T // NCORES  # tokens per core in router phase
HO = H // P  # 8
FO = F // P  # 16
F32 = mybir.dt.float32
BF16 = mybir.dt.bfloat16
BF16_NP = ml_dtypes.bfloat16

_NC_CACHE = {}
_W_CACHE = {}


def _token_chunks(C):
    """Split C into free-dim chunks <= 512 (>= 256 when C >= 512)."""
    assert C >= 1
    chunks = []
    t0 = 0
    while t0 < C:
        rem = C - t0
        if rem >= 768:
            tl = 512
        elif rem > 512:
            tl = rem - 256  # leaves a final 256 chunk; both >= 256
        else:
            tl = rem
        chunks.append((t0, tl))
        t0 += tl
    return chunks


def _build_router():
    nc = bacc.Bacc("TRN2", target_bir_lowering=False, debug=False,
                   enable_partition_id=False)
    xT = nc.dram_tensor("xT", [H, TSH], F32, kind="ExternalInput")
    rwt = nc.dram_tensor("rwt", [P, HO, E], F32, kind="ExternalInput")
    c_out = nc.dram_tensor("c", [TSH, E], F32, kind="ExternalOutput")
    TT = TSH // P  # 4 token blocks
    with tile.TileContext(nc) as tc:
        with tc.tile_pool(name="xp", bufs=1) as xp, \
             tc.tile_pool(name="wp", bufs=1) as wp, \
             tc.tile_pool(name="sp", bufs=4) as sp, \
             tc.tile_pool(name="ps", bufs=4, space="PSUM") as ps:
            # rw is tiny and only truly needed before the final matmul
            # burst (~9us; the chase matmuls have slack), so it rides late.
            rw = wp.tile([P, HO, E], F32)
            xhos = []
            for ho in range(HO):
                xh = xp.tile([P, TSH], F32, tag=f"xh{ho}", name=f"xh{ho}")
                # Both HWDGE queues (sync + scalar); gpsimd's SWDGE pays
                # ~1us generation per DMA and starves the PE.
                dma = nc.sync.dma_start if ho % 2 == 0 else nc.scalar.dma_start
                dma(xh[:], xT.ap()[ho * P:(ho + 1) * P, :])
                xhos.append(xh)
                # slot 7: by here the wire runs fully gen-ahead, so rw's
                # tiny copy slots in with zero pipeline break, and its
                # ~8.9us deadline (final matmul burst) is still met.
                if ho == 5:
                    nc.sync.dma_start(rw[:], rwt.ap())
            pls = [ps.tile([P, E], F32, tag="pl", name=f"pl{tt}")
                   for tt in range(TT)]
            # tt-outer: pl[tt] stops right after the last x row-block lands,
            # so the top-2 chains pipeline behind the remaining matmuls.
            for tt in range(TT):
                for ho in range(HO):
                    nc.tensor.matmul(pls[tt][:],
                                     xhos[ho][:, tt * P:(tt + 1) * P],
                                     rw[:, ho, :],
                                     start=(ho == 0), stop=(ho == HO - 1))
            # One coalesced c store at the end: each extra DMA costs ~625ns
            # HWDGE generation + 900ns semaphore propagation on the tail.
            cga = sp.tile([P, TT, E], F32, tag="cga")
            for tt in range(TT):
                # c = z / (z1 + z2) over top-2 z; z = exp(logit) (shift-free:
                # |logit| <= ~5 so exp stays in fp32 range).
                z = sp.tile([P, E], F32, tag="z")
                nc.scalar.activation(z[:], pls[tt][:],
                                     mybir.ActivationFunctionType.Exp)
                m8 = sp.tile([P, 8], F32, tag="m8")
                nc.vector.max(m8[:], z[:])
                s2 = sp.tile([P, 1], F32, tag="s2")
                nc.vector.tensor_reduce(s2[:], m8[:, 0:2],
                                        axis=mybir.AxisListType.X,
                                        op=mybir.AluOpType.add)
                rec = sp.tile([P, 1], F32, tag="rec")
                nc.vector.reciprocal(rec[:], s2[:])
                msk = sp.tile([P, E], F32, tag="msk")
                nc.vector.tensor_scalar(msk[:], z[:], m8[:, 1:2], rec[:, 0:1],
                                        op0=mybir.AluOpType.is_ge,
                                        op1=mybir.AluOpType.mult)
                nc.vector.tensor_mul(cga[:, tt, :], z[:], msk[:])
            nc.sync.dma_start(
                c_out.ap().rearrange("(tt p) e -> p tt e", p=P), cga[:])
    nc.compile()
    return nc


def _build_expert(C, CA=None):
    if CA is None:
        CA = C
    assert 1 <= CA <= C
    CA = min(C, CA)  # exact active count; bf16 needs no alignment pad
    chunks = _token_chunks(CA)
    assert len(chunks) <= 3  # psum budget: 2 tags x 3 bufs + 2 = 8 banks
    nc = bacc.Bacc("TRN2", target_bir_lowering=False, debug=False,
                   enable_partition_id=False)
    xgT = nc.dram_tensor("xgT", [H, C], BF16, kind="ExternalInput")
    cbt = nc.dram_tensor("cbt", [P, C], F32, kind="ExternalInput")
    w1t = nc.dram_tensor("w1t", [FO, P, HO, P], BF16, kind="ExternalInput")
    v1t = nc.dram_tensor("v1t", [FO, P, HO, P], BF16, kind="ExternalInput")
    w2t = nc.dram_tensor("w2t", [HO, P, FO, P], BF16, kind="ExternalInput")
    # bf16 output (host upcasts): halves store wire and the final-store tail.
    yT = nc.dram_tensor("yT", [H, C], BF16, kind="ExternalOutput")
    with tile.TileContext(nc) as tc:
        with tc.tile_pool(name="xp", bufs=1) as xp, \
             tc.tile_pool(name="hp", bufs=1) as hp, \
             tc.tile_pool(name="wp", bufs=2) as wp, \
             tc.tile_pool(name="w2p", bufs=1) as w2p, \
             tc.tile_pool(name="cp", bufs=1) as cp, \
             tc.tile_pool(name="yp", bufs=4) as yp, \
             tc.tile_pool(name="ps", bufs=3, space="PSUM") as ps, \
             tc.tile_pool(name="psb", bufs=2, space="PSUM") as psb:
            HH = HO // 2

            def load_wv_half(fo, half):
                w1s = wp.tile([P, HH, P], BF16, tag=f"w1s{half}",
                              name=f"w1s{fo}_{half}")
                nc.sync.dma_start(
                    w1s[:], w1t.ap()[fo, :, half * HH:(half + 1) * HH, :])
                v1s = wp.tile([P, HH, P], BF16, tag=f"v1s{half}",
                              name=f"v1s{fo}_{half}")
                nc.sync.dma_start(
                    v1s[:], v1t.ap()[fo, :, half * HH:(half + 1) * HH, :])
                return (w1s, v1s)

            def load_wv(fo):
                return [load_wv_half(fo, 0), load_wv_half(fo, 1)]

            def wsl(halves, mat, ho):
                return halves[ho // HH][mat][:, ho % HH, :]

            # PE p-state warmup: the sim (and HW) clock the PE at 0.65/1.2
            # GHz until ~3us of continuous execution.  Burn the ramp on
            # fine-grained dummy matmuls over a zeroed tile while the first
            # DMAs land (~3.7us), so the real matmuls start at full clock.
            zmm = xp.tile([P, 256], BF16, tag="zmm")
            nc.vector.memset(zmm[:].bitcast(mybir.dt.uint16), 0)
            warm = psb.tile([P, 512], F32, tag="psy", name="warm")
            for _ in range(13):
                nc.tensor.matmul(warm[:, :256], zmm[:, :P], zmm[:],
                                 start=True, stop=True)
            # Sync/HWDGE queue carries only the w1/v1 stream; xg + cbs + w2
            # ride the gpsimd SWDGE whose descriptor generator runs in
            # parallel with HWDGE's (one shared HWDGE generator otherwise
            # serializes everything at ~625ns per DMA).  Only fo=0's first
            # half leads the xg stream; the other early halves queue after
            # so xg1/xg2 wires land before the prologue needs them.
            wv0 = [load_wv_half(0, 0), None]
            xgs = []
            for ho in range(HO):
                xgc = xp.tile([P, CA], BF16, tag=f"xg{ho}", name=f"xg{ho}")
                nc.gpsimd.dma_start(xgc[:],
                                    xgT.ap()[ho * P:(ho + 1) * P, :CA])
                xgs.append(xgc)

            def xg(ho, t0, tl):
                return xgs[ho][:, t0:t0 + tl]
            # Remaining early weight tiles in need-order, w1 paths first:
            # wv0's v1-half1 (first used at ho=4) and fo=1's v1 halves
            # (first used at fo=1) wire AFTER the critical xg row-blocks.
            w1s01 = wp.tile([P, HH, P], BF16, tag="w1s1", name="w1s0_1")
            nc.sync.dma_start(w1s01[:], w1t.ap()[0, :, HH:, :])
            w1s1h = []
            for half in range(2):
                w1s = wp.tile([P, HH, P], BF16, tag=f"w1s{half}",
                              name=f"w1s1_{half}")
                nc.sync.dma_start(
                    w1s[:], w1t.ap()[1, :, half * HH:(half + 1) * HH, :])
                w1s1h.append(w1s)
            v1s01 = wp.tile([P, HH, P], BF16, tag="v1s1", name="v1s0_1")
            nc.sync.dma_start(v1s01[:], v1t.ap()[0, :, HH:, :])
            wv0[1] = (w1s01, v1s01)
            v1s1h = []
            for half in range(2):
                v1s = wp.tile([P, HH, P], BF16, tag=f"v1s{half}",
                              name=f"v1s1_{half}")
                nc.sync.dma_start(
                    v1s[:], v1t.ap()[1, :, half * HH:(half + 1) * HH, :])
                v1s1h.append(v1s)
            wv1 = [(w1s1h[0], v1s1h[0]), (w1s1h[1], v1s1h[1])]
            # w2 preloaded during Phase A (plenty of DMA slack: Phase A
            # wire ~33 MB << 358 GB/s * 115 us).
            w2s = []
            for hg in range(HO):
                w2g = w2p.tile([P, FO, P], BF16, tag=f"w2s{hg}",
                               name=f"w2s{hg}")
                nc.gpsimd.dma_start(w2g[:], w2t.ap()[hg])
                w2s.append(w2g)
            # cbs is first needed at Phase B's gate (~120us); issuing it
            # last keeps its 1.5us fp32 wire out of the prologue window
            # where it displaced the late xg row-blocks and fo2 weights.
            cbs = cp.tile([P, CA], F32)
            nc.gpsimd.dma_start(cbs[:], cbt.ap()[:, :CA])
            h = hp.tile([P, FO, CA], BF16)

            def glu_tail(fo, t0, tl, p1, p2):
                hs = h[:, fo, t0:t0 + tl]
                nc.scalar.activation(hs, p1,
                                     mybir.ActivationFunctionType.Silu)
                nc.vector.tensor_mul(hs, hs, p2)

            # Phase A: h = silu(w1t^T xg) * (v1t^T xg), laid out [f, t].
            # Prologue runs fo=0 (all chunks) plus ONLY the w1-path of
            # fo=1's chunk0 (pre1): per-ho consumption (~1.10us) matches
            # the xg arrival rate, and fo=1's v1-half wire (the profiled
            # 1.3us prologue stall) drops out of the early window.  fo=1
            # computes the v1-path of chunk0 itself and pairs it with the
            # held pre1 psum in its glu tail.
            t00, tl0 = chunks[0]
            ps1s = [ps.tile([P, 512], F32, tag="ps1", name=f"ps1_{i}")[:, :tl]
                    for i, (t0, tl) in enumerate(chunks)]
            ps2s = [ps.tile([P, 512], F32, tag="ps2", name=f"ps2_{i}")[:, :tl]
                    for i, (t0, tl) in enumerate(chunks)]
            pre1 = psb.tile([P, 512], F32, tag="psy", name="pre1")[:, :tl0]
            for ho in range(HO):
                st, sp_ = (ho == 0), (ho == HO - 1)
                for i, (t0, tl) in enumerate(chunks):
                    nc.tensor.matmul(ps1s[i], wsl(wv0, 0, ho),
                                     xg(ho, t0, tl),
                                     start=st, stop=sp_)
                    nc.tensor.matmul(ps2s[i], wsl(wv0, 1, ho),
                                     xg(ho, t0, tl),
                                     start=st, stop=sp_)
                nc.tensor.matmul(pre1, wsl(wv1, 0, ho),
                                 xg(ho, t00, tl0),
                                 start=st, stop=sp_)
            for i, (t0, tl) in enumerate(chunks):
                glu_tail(0, t0, tl, ps1s[i], ps2s[i])

            for fo in range(1, FO):
                wv = wv1 if fo == 1 else load_wv(fo)
                w1_chunks = (list(enumerate(chunks))[1:] if fo == 1
                             else list(enumerate(chunks)))
                ps1s = [ps.tile([P, 512], F32, tag="ps1",
                                name=f"ps1_{i}")[:, :tl]
                        for i, (t0, tl) in w1_chunks]
                ps2s = [ps.tile([P, 512], F32, tag="ps2",
                                name=f"ps2_{i}")[:, :tl]
                        for i, (t0, tl) in enumerate(chunks)]
                for ho in range(HO):
                    st, sp_ = (ho == 0), (ho == HO - 1)
                    for j, (i, (t0, tl)) in enumerate(w1_chunks):
                        nc.tensor.matmul(ps1s[j], wsl(wv, 0, ho),
                                         xg(ho, t0, tl),
                                         start=st, stop=sp_)
                    for i, (t0, tl) in enumerate(chunks):
                        nc.tensor.matmul(ps2s[i], wsl(wv, 1, ho),
                                         xg(ho, t0, tl),
                                         start=st, stop=sp_)
                if fo == 1:
                    glu_tail(1, t00, tl0, pre1, ps2s[0])
                    for j, (i, (t0, tl)) in enumerate(w1_chunks):
                        glu_tail(fo, t0, tl, ps1s[j], ps2s[i])
                else:
                    for j, (i, (t0, tl)) in enumerate(w1_chunks):
                        glu_tail(fo, t0, tl, ps1s[j], ps2s[i])

            # Phase B (H-major): yT[hg*128:+128, t] = sum_f w2[f, h] h[f, t],
            # gated per token column by cbs.  Streams exactly CA columns.
            # chunk0 first: its h completes first at fo=15, so Phase B
            # starts with no boundary stall.
            for ci, (t0, tl) in enumerate(chunks):
                for hg in range(HO):
                    py = psb.tile([P, 512], F32, tag="psy", name="py")[:, :tl]
                    for fo in range(FO):
                        nc.tensor.matmul(py, w2s[hg][:, fo, :],
                                         h[:, fo, t0:t0 + tl],
                                         start=(fo == 0),
                                         stop=(fo == FO - 1))
                    yt = yp.tile([P, 512], BF16, tag="yt", name="yt")[:, :tl]
                    nc.vector.tensor_mul(yt, py, cbs[:, t0:t0 + tl])
                    # sync + scalar HWDGE queues are idle in Phase B.
                    dma = (nc.sync.dma_start if hg % 2 == 0
                           else nc.scalar.dma_start)
                    dma(yT.ap()[hg * P:(hg + 1) * P, t0:t0 + tl], yt)
    nc.compile()
    return nc


def _get_nc(key, builder):
    if key not in _NC_CACHE:
        _NC_CACHE[key] = builder()
    return _NC_CACHE[key]


def _tile_weights(w1, v1, w2):
    """Pre-tile the expert weights (bf16) for large-descriptor DMA.

    w1t/v1t: [E, FO, 128(h), HO, 128(f)]  (lhsT tiles of [H,F] transposed)
    w2t:     [E, HO(hg), 128(f_lo), FO, 128(h)]  (lhsT tiles of w2 [F, H])
    """
    key = (w1.shape, w1.dtype.str, w1[0, 0, :4].tobytes(),
           w2[0, 0, :4].tobytes(), v1[0, 0, :4].tobytes(),
           float(w1[-1, -1, -1]), float(w2[-1, -1, -1]))
    if key in _W_CACHE:
        return _W_CACHE[key]
    # w1[e] is [F, H]; lhsT tile (fo): [p_h, ho, q_f] = w1[e][fo*128+q, ho*128+p]
    w1t = np.ascontiguousarray(
        w1.reshape(E, FO, P, HO, P).transpose(0, 1, 4, 3, 2)).astype(BF16_NP)
    v1t = np.ascontiguousarray(
        v1.reshape(E, FO, P, HO, P).transpose(0, 1, 4, 3, 2)).astype(BF16_NP)
    # w2[e] is [F, H]; lhsT tile (hg, fo): [p_f, i_h] = w2[e][fo*128+p, hg*128+i]
    w2t = np.ascontiguousarray(
        w2.reshape(E, FO, P, HO, P).transpose(0, 3, 2, 1, 4)).astype(BF16_NP)
    _W_CACHE.clear()
    _W_CACHE[key] = (w1t, v1t, w2t)
    return w1t, v1t, w2t


def kernel(x, router_w, w1, v1, w2):
    x = np.asarray(x, dtype=np.float32)
    router_w = np.asarray(router_w, dtype=np.float32)
    w1 = np.asarray(w1, dtype=np.float32)
    v1 = np.asarray(v1, dtype=np.float32)
    w2 = np.asarray(w2, dtype=np.float32)

    xf = x.reshape(T, H)
    xT = np.ascontiguousarray(xf.T)  # [H, T]
    # rwt[p, ho, e] = router_w[e, ho*128+p]
    rwt = np.ascontiguousarray(
        router_w.T.reshape(HO, P, E).transpose(1, 0, 2))

    # ---- Phase 1: router on device (data-parallel over tokens) ----
    nc1 = _get_nc("router", _build_router)
    in1 = [{"xT": np.ascontiguousarray(xT[:, i * TSH:(i + 1) * TSH]),
            "rwt": rwt}
           for i in range(NCORES)]
    r1 = run_bass_kernel_spmd(nc1, in1, core_ids=list(range(NCORES)))
    c = np.concatenate([r["c"] for r in r1.results], axis=0)  # [T, E]

    # ---- Host dispatch: gather tokens per expert (data movement only) ----
    xTb = xT.astype(BF16_NP)  # bf16 activations for the expert phase
    idxs = [np.flatnonzero(c[:, e] != 0.0) for e in range(E)]
    maxc = max(len(ix) for ix in idxs)
    # Per-launch capacity; >1280 tokens per expert (never happens with
    # balanced routing) is handled by running the same NEFF multiple times.
    C = max(1152, min(1280, ((maxc + 127) // 128) * 128))
    nseg = (maxc + C - 1) // C

    w1t, v1t, w2t = _tile_weights(w1, v1, w2)

    out = np.zeros((T, H), np.float32)
    for seg in range(nseg):
        segixs = [idxs[e][seg * C:(seg + 1) * C] for e in range(E)]
        CA = max(1, max(len(ix) for ix in segixs))  # exact active count
        nc2 = _get_nc(("expert", C, CA), lambda: _build_expert(C, CA))
        in2 = []
        for e in range(E):
            ix = segixs[e]
            xgT = np.zeros((H, C), BF16_NP)
            xgT[:, :len(ix)] = xTb[:, ix]
            cge = np.zeros((C,), np.float32)
            cge[:len(ix)] = c[ix, e]
            cbt = np.ascontiguousarray(
                np.broadcast_to(cge[None, :], (P, C)))
            in2.append({"xgT": xgT, "cbt": cbt,
                        "w1t": w1t[e], "v1t": v1t[e], "w2t": w2t[e]})
        r2 = run_bass_kernel_spmd(nc2, in2, core_ids=list(range(NCORES)))
        # ---- Host combine: scatter-add per-expert outputs ----
        for e in range(E):
            ix = segixs[e]
            out[ix] += r2.results[e]["yT"][:, :len(ix)].T.astype(np.float32)
    return out.reshape(x.shape)
